# revision 1
# baseline (speedup 1.0000x reference)
"""Trainium2 Bass kernel for nn_Actor_1580547975181 (segment_reduce).

out[n, a] = log_softmax(action_select)[n, a] + scatter_log_softmax(device_select)[n]
  action_select = (x_r @ Wa_r.T + x_i @ Wa_i.T) / 128     [N, 6]
  device_select = (x_r @ Wd_r[0] + x_i @ Wd_i[0]) / 128   [N]
  groups = sorted batch_index (B=8192 segments over N=1048576 rows)

Sharding: rows split across 8 cores at group boundaries (batch_index sorted);
each core gets a fixed window of NP=133120 rows (windows overlap; only the
group-aligned [start_c, start_{c+1}) range of each core's output is used).
Weights are tiny, replicated, pre-packed to bf16 [128, 8] on host.

Device pipeline per core (all compute on device):
  * chunks of 4096 rows (32 tiles): SWDGE cast-DMA f32->bf16 (HBM read at
    line rate, 512B descriptors in 32 interleaved sequential streams) ->
    one SBUF->SBUF xbar transpose per chunk ([128, 64, 128]; plane t is the
    transposed 128-row tile t, r-array planes then i-array planes) -> per
    tile two K=128 matmuls (xr@W8r + xi@W8i accumulated in PSUM) ->
    y [128 rows, 32 tiles, 8] (cols 0:6 action, 6 device, 7 pad).
  * chunk epilogue: exp over the 6 actions (max-shift unnecessary: selects
    are O(0.03)), per-row action exp-sums and raw bf16 y_a stashed; ln is
    deferred and batched so the ACT engine never swaps function tables.
  * per slab (16384 rows = 128 tiles; one 2048-row 16-tile tail slab):
    PE-transpose v into scan layout [tile-part, row-free]; segmented scan of
    e = exp(v) with reset masks from batch_index (state = mask*state + e);
    carry propagation across partitions via a [1, tps] scan chained across
    slabs; reversed segmented scan fills each row with its group total G.
  * one global backward pass fixes fills for groups crossing partition
    starts (groups span at most 2 partitions).
  * final: out = stash + (v - ln(asum) - ln G) per row, written [NP, 8] f32
    (host keeps cols 0:6 of the valid row range).
"""

from contextlib import ExitStack

import numpy as np
import ml_dtypes

import concourse.bass as bass
import concourse.tile as tile
from concourse import bacc, mybir
from concourse.bass_utils import run_bass_kernel_spmd

F32 = mybir.dt.float32
BF16 = mybir.dt.bfloat16
I32 = mybir.dt.int32
AF = mybir.ActivationFunctionType
OP = mybir.AluOpType
AX = mybir.AxisListType

N_CORES = 8
DIM = 128
N_TOTAL = 1048576
CH = 64                                   # tiles per (full) chunk
ROWS_PER_TILE = 128
ROWS_PER_CHUNK = CH * ROWS_PER_TILE       # 8192
ROWS_PER_SLAB = 128 * ROWS_PER_TILE       # 16384
NP = 133120                               # 8 full slabs + one 2048-row tail


def _bcast(ap2, n):
    return bass.AP(ap2.tensor, ap2.offset, [*[list(p) for p in ap2.ap], [0, n]])


def build(NP, n_devices=N_CORES):
    tail = NP % ROWS_PER_SLAB
    assert tail in (0, 2048)
    nfull = NP // ROWS_PER_SLAB
    nslab = nfull + (1 if tail else 0)
    ntile = NP // ROWS_PER_TILE
    nflat = nslab * 128
    cps = ROWS_PER_SLAB // ROWS_PER_CHUNK

    nc = bacc.Bacc("TRN2", target_bir_lowering=False, debug=False,
                   num_devices=n_devices)

    xr = nc.dram_tensor("xr", [NP, 128], F32, kind="ExternalInput")
    xi = nc.dram_tensor("xi", [NP, 128], F32, kind="ExternalInput")
    w8r = nc.dram_tensor("w8r", [128, 8], BF16, kind="ExternalInput")
    w8i = nc.dram_tensor("w8i", [128, 8], BF16, kind="ExternalInput")
    bip = nc.dram_tensor("bip", [NP + 2], I32, kind="ExternalInput")
    ident = nc.dram_tensor("ident", [128, 128], F32, kind="ExternalInput")
    out8 = nc.dram_tensor("out8", [NP, 8], F32, kind="ExternalOutput")

    with tile.TileContext(nc) as tc, ExitStack() as ctx:
        ep = ctx.enter_context
        pconst = ep(tc.tile_pool(name="const", bufs=1))
        ppers = ep(tc.tile_pool(name="pers", bufs=1))
        pstg = ep(tc.tile_pool(name="stg", bufs=2))
        pxt = ep(tc.tile_pool(name="xt", bufs=2))
        pscr = ep(tc.tile_pool(name="scr", bufs=3))
        pslab = ep(tc.tile_pool(name="slab", bufs=2))
        pbi = ep(tc.tile_pool(name="bi", bufs=2))
        pout = ep(tc.tile_pool(name="outp", bufs=3))
        pflat = ep(tc.tile_pool(name="flat", bufs=1))
        ppsY = ep(tc.tile_pool(name="psY", bufs=4, space="PSUM"))
        ppsT = ep(tc.tile_pool(name="psT", bufs=2, space="PSUM"))
        ppsS = ep(tc.tile_pool(name="psS", bufs=2, space="PSUM"))

        idt = pconst.tile([128, 128], F32)
        nc.sync.dma_start(idt[:], ident.ap())
        wr_t = pconst.tile([128, 8], BF16)
        nc.sync.dma_start(wr_t[:], w8r.ap())
        wi_t = pconst.tile([128, 8], BF16)
        nc.sync.dma_start(wi_t[:], w8i.ap())

        stash = ppers.tile([128, ntile * 6], BF16)
        vstage = ppers.tile([128, ntile], F32)
        Gall = ppers.tile([128, nflat], F32)
        asumstage = ppers.tile([128, ntile], F32)
        contpack = ppers.tile([128, nslab], F32)
        fullpack = ppers.tile([128, nslab], F32)
        G0pack = ppers.tile([128, nslab], F32)
        Fpack = ppers.tile([128, nslab], F32)
        chain = ppers.tile([1, nslab + 1], F32)
        nc.vector.memset(chain[:], 0.0)
        nc.vector.memset(contpack[:], 0.0)
        nc.vector.memset(fullpack[:], 0.0)
        nc.vector.memset(G0pack[:], 0.0)

        def emit_chunk(row0, ch):
            # staging: per partition p (= row-within-tile) the ch tiles' rows
            # sit side by side (512B DRAM reads, contiguous SBUF writes);
            # transposed plane t is then exactly tile t's [d, row].
            tb = row0 // 128
            srcR = bass.AP(xr, row0 * 128,
                           [[128, 128], [128 * 128, ch], [1, 128]])
            srcI = bass.AP(xi, row0 * 128,
                           [[128, 128], [128 * 128, ch], [1, 128]])
            stA = pstg.tile([128, 2 * CH * 128], BF16)
            nc.gpsimd.dma_start(stA[:, 0:ch * 128], srcR)
            nc.gpsimd.dma_start(stA[:, ch * 128:2 * ch * 128], srcI)
            xtA = pxt.tile([128, 2 * CH, 128], BF16)
            nc.sync.dma_start_transpose(xtA[:, 0:2 * ch, :],
                                        stA[:, 0:2 * ch * 128])
            y = ppsY.tile([128, CH, 8], F32, tag="y")
            for t in range(ch):
                nc.tensor.matmul(y[:, t, :], xtA[:, t, :], wr_t[:],
                                 start=True, stop=False)
                nc.tensor.matmul(y[:, t, :], xtA[:, ch + t, :], wi_t[:],
                                 start=False, stop=True)
            ea = pscr.tile([128, CH * 6], F32)
            ea3 = ea[:, 0:ch * 6].rearrange("p (t a) -> p t a", a=6)
            nc.scalar.activation(ea3, y[:, 0:ch, 0:6], AF.Exp)
            nc.vector.tensor_reduce(asumstage[:, tb:tb + ch], ea3,
                                    axis=AX.X, op=OP.add)
            st3 = stash[:, tb * 6:(tb + ch) * 6].rearrange(
                "p (t a) -> p t a", a=6)
            nc.vector.tensor_copy(st3, y[:, 0:ch, 0:6])
            nc.vector.tensor_copy(vstage[:, tb:tb + ch], y[:, 0:ch, 6])

        def load_bi(s, tps):
            bi3 = pbi.tile([128, 130], I32)
            nc.scalar.dma_start(
                bi3[0:tps, :],
                bass.AP(bip, s * ROWS_PER_SLAB, [[128, tps], [1, 130]]))
            return bi3

        def emit_slab_fwd(s, tps):
            tb = s * 128  # tile base (y-layout cols)
            bi3 = load_bi(s, tps)
            bi = bi3[0:tps, 1:129]
            bprev = bi3[0:tps, 0:128]
            bnext = bi3[0:tps, 2:130]

            vT = ppsT.tile([128, 128], F32, tag="tp128")
            nc.tensor.transpose(vT[0:tps, :], vstage[:, tb:tb + tps], idt[:])
            escan = pslab.tile([128, 128], F32, tag="escan")
            nc.scalar.activation(escan[0:tps, :], vT[0:tps, :], AF.Exp)

            maskC = pslab.tile([128, 128], F32, tag="maskC")
            nc.vector.tensor_tensor(maskC[0:tps, :], bi, bprev, op=OP.is_equal)
            S = pslab.tile([128, 128], F32, tag="S")
            nc.vector.tensor_tensor_scan(S[0:tps, :], maskC[0:tps, :],
                                         escan[0:tps, :], 0.0,
                                         op0=OP.mult, op1=OP.add)

            pack = pslab.tile([128, 4], F32, tag="pack")
            nc.vector.tensor_copy(pack[0:tps, 0:1], S[0:tps, 127:128])
            nc.vector.tensor_tensor(pack[0:tps, 2:3], bi3[0:tps, 1:2],
                                    bi3[0:tps, 0:1], op=OP.is_equal)
            nc.vector.tensor_tensor(pack[0:tps, 3:4], bi3[0:tps, 1:2],
                                    bi3[0:tps, 128:129], op=OP.is_equal)
            nc.vector.tensor_tensor(pack[0:tps, 1:2], pack[0:tps, 2:3],
                                    pack[0:tps, 3:4], op=OP.mult)
            packT_ps = ppsS.tile([4, 128], F32, tag="tpS")
            nc.tensor.transpose(packT_ps[:, 0:tps], pack[0:tps, :],
                                idt[0:tps, 0:tps])
            lrow = pslab.tile([4, 128], F32, tag="lrow")
            nc.vector.tensor_copy(lrow[:, 0:tps], packT_ps[:, 0:tps])
            # engines can't read base-partition 1/2 — bounce rows via DMA
            arow = pslab.tile([1, 128], F32, tag="arow")
            nc.sync.dma_start(arow[:, 0:tps], lrow[1:2, 0:tps])
            crow = pslab.tile([1, 128], F32, tag="crow")
            nc.sync.dma_start(crow[:, 0:tps], lrow[2:3, 0:tps])

            strow = pslab.tile([1, 129], F32, tag="strow")
            nc.vector.tensor_tensor_scan(strow[:, 0:tps], arow[:, 0:tps],
                                         lrow[0:1, 0:tps], chain[:, s:s + 1],
                                         op0=OP.mult, op1=OP.add)
            nc.vector.tensor_copy(chain[:, s + 1:s + 2],
                                  strow[:, tps - 1:tps])
            cprow = pslab.tile([1, 128], F32, tag="cprow")
            if tps > 1:
                nc.vector.tensor_copy(cprow[:, 1:tps], strow[:, 0:tps - 1])
            nc.vector.tensor_copy(cprow[:, 0:1], chain[:, s:s + 1])
            cinrow = pslab.tile([1, 128], F32, tag="cinrow")
            nc.vector.tensor_tensor(cinrow[:, 0:tps], crow[:, 0:tps],
                                    cprow[:, 0:tps], op=OP.mult)
            cin_ps = ppsS.tile([128, 1], F32, tag="tpS")
            nc.tensor.transpose(cin_ps[0:tps, :], cinrow[:, 0:tps],
                                idt[0:1, 0:1])
            cin = pslab.tile([128, 1], F32, tag="cin")
            nc.vector.tensor_copy(cin[0:tps, :], cin_ps[0:tps, :])

            bif = pslab.tile([128, 129], F32, tag="bif")
            nc.vector.tensor_copy(bif[0:tps, :], bi3[0:tps, 1:130])
            m1 = pslab.tile([128, 128], F32, tag="m1")
            nc.vector.tensor_scalar(m1[0:tps, :], bif[0:tps, 0:128],
                                    bif[0:tps, 0:1], None, op0=OP.is_equal)
            Sf = pslab.tile([128, 128], F32, tag="Sf")
            nc.vector.scalar_tensor_tensor(Sf[0:tps, :], m1[0:tps, :],
                                           cin[0:tps, :], S[0:tps, :],
                                           op0=OP.mult, op1=OP.add)

            flagE = pslab.tile([128, 128], F32, tag="flagE")
            nc.vector.tensor_tensor(flagE[0:tps, :], bi, bnext,
                                    op=OP.not_equal)
            notE = pslab.tile([128, 128], F32, tag="notE")
            nc.vector.tensor_tensor(notE[0:tps, :], bi, bnext, op=OP.is_equal)
            t1 = pslab.tile([128, 128], F32, tag="t1")
            nc.vector.tensor_tensor(t1[0:tps, :], flagE[0:tps, :],
                                    Sf[0:tps, :], op=OP.mult)
            G = Gall[:, s * 128:(s + 1) * 128]
            nc.vector.tensor_tensor_scan(G[0:tps, ::-1], notE[0:tps, ::-1],
                                         t1[0:tps, ::-1], 0.0,
                                         op0=OP.mult, op1=OP.add)
            nc.vector.tensor_copy(G0pack[0:tps, s:s + 1], G[0:tps, 0:1])
            nc.vector.tensor_copy(contpack[0:tps, s:s + 1], pack[0:tps, 2:3])
            nc.vector.tensor_copy(fullpack[0:tps, s:s + 1], pack[0:tps, 3:4])

        def emit_global_bwd():
            # flatten [128, nslab] packs to [1, nflat] rows (global tile
            # order); tail-slab's unused partitions were memset to 0 so the
            # backward chain terminates there.
            fa = pflat.tile([1, nflat], F32, tag="fa")   # cont -> D2
            fb = pflat.tile([1, nflat], F32, tag="fb")   # full -> A2
            fc = pflat.tile([1, nflat], F32, tag="fc")   # g0   -> Ffl
            fd = pflat.tile([1, nflat], F32, tag="fd")   # U / V scratch
            for fl, srcp in ((fa, contpack), (fb, fullpack), (fc, G0pack)):
                tps_ = ppsS.tile([nslab, 128], F32, tag="tpS")
                nc.tensor.transpose(tps_[:], srcp[:], idt[:])
                sq = pscr.tile([nslab, 128], F32, tag="sq")
                nc.vector.tensor_copy(sq[:], tps_[:])
                nc.sync.dma_start(fl[:], sq[:])
            nc.vector.tensor_tensor(fd[:], fa[:], fb[:], op=OP.mult)
            nc.vector.tensor_copy(fb[:, 0:nflat - 1], fd[:, 1:nflat])
            nc.vector.memset(fb[:, nflat - 1:nflat], 0.0)
            nc.vector.tensor_tensor(fd[:], fa[:], fc[:], op=OP.mult)
            nc.vector.tensor_copy(fa[:, 0:nflat - 1], fd[:, 1:nflat])
            nc.vector.memset(fa[:, nflat - 1:nflat], 0.0)
            nc.vector.tensor_tensor_scan(fc[:, ::-1], fb[:, ::-1],
                                         fa[:, ::-1], 0.0,
                                         op0=OP.mult, op1=OP.add)
            Fsq = pscr.tile([nslab, 128], F32, tag="Fsq")
            ffl_ap = fc[:]
            nc.sync.dma_start(
                Fsq[:], bass.AP(ffl_ap.tensor, ffl_ap.offset,
                                [[ffl_ap.ap[0][0], 1], [128, nslab], [1, 128]]))
            fps = ppsS.tile([128, nslab], F32, tag="tpS")
            nc.tensor.transpose(fps[:], Fsq[:], idt[0:nslab, 0:nslab])
            nc.vector.tensor_copy(Fpack[:], fps[:])

        def emit_slab_final(s, tps):
            tb = s * 128
            bi3 = load_bi(s, tps)
            G = Gall[:, s * 128:(s + 1) * 128]
            bif = pslab.tile([128, 129], F32, tag="bif2")
            nc.vector.tensor_copy(bif[0:tps, :], bi3[0:tps, 1:130])
            m2 = pslab.tile([128, 128], F32, tag="m2")
            nc.vector.tensor_scalar(m2[0:tps, :], bif[0:tps, 0:128],
                                    bif[0:tps, 127:128], None,
                                    op0=OP.is_equal)
            Gf = pslab.tile([128, 128], F32, tag="Gf")
            nc.vector.scalar_tensor_tensor(Gf[0:tps, :], m2[0:tps, :],
                                           Fpack[0:tps, s:s + 1], G[0:tps, :],
                                           op0=OP.mult, op1=OP.add)
            lg = pslab.tile([128, 128], F32, tag="lg")
            nc.scalar.activation(lg[0:tps, :], Gf[0:tps, :], AF.Ln)
            lgT_ps = ppsT.tile([128, 128], F32, tag="tp128")
            nc.tensor.transpose(lgT_ps[:, 0:tps], lg[0:tps, :],
                                idt[0:tps, 0:tps])
            lnas = pslab.tile([128, 128], F32, tag="lnas")
            nc.scalar.activation(lnas[:, 0:tps], asumstage[:, tb:tb + tps],
                                 AF.Ln)
            rdiff = pslab.tile([128, 128], F32, tag="rdiff")
            nc.vector.tensor_tensor(rdiff[:, 0:tps], vstage[:, tb:tb + tps],
                                    lnas[:, 0:tps], op=OP.subtract)
            nc.vector.tensor_tensor(rdiff[:, 0:tps], rdiff[:, 0:tps],
                                    lgT_ps[:, 0:tps], op=OP.subtract)
            nch = (tps * 128 + ROWS_PER_CHUNK - 1) // ROWS_PER_CHUNK
            for c in range(nch):
                row0 = s * ROWS_PER_SLAB + c * ROWS_PER_CHUNK
                ch = min(CH, tps - c * CH)
                tbc = row0 // 128
                ob = pout.tile([128, CH, 8], F32)
                st3 = stash[:, tbc * 6:(tbc + ch) * 6].rearrange(
                    "p (t a) -> p t a", a=6)
                rsl = rdiff[:, c * CH:c * CH + ch]
                nc.vector.tensor_tensor(ob[:, 0:ch, 0:6], st3,
                                        _bcast(rsl, 6), op=OP.add)
                dst = bass.AP(out8, row0 * 8,
                              [[8, 128], [128 * 8, ch], [1, 8]])
                nc.scalar.dma_start(dst, ob[:, 0:ch, :])

        slabs = [(s, 128) for s in range(nfull)]
        if tail:
            slabs.append((nfull, tail // ROWS_PER_TILE))
        for s, tps in slabs:
            nch = (tps * 128 + ROWS_PER_CHUNK - 1) // ROWS_PER_CHUNK
            for c in range(nch):
                row0 = s * ROWS_PER_SLAB + c * ROWS_PER_CHUNK
                ch = min(CH, tps - c * CH)
                emit_chunk(row0, ch)
            emit_slab_fwd(s, tps)
        emit_global_bwd()
        for s, tps in slabs:
            emit_slab_final(s, tps)

    nc.compile()
    return nc


_NC_CACHE = {}


def _get_nc():
    if "nc" not in _NC_CACHE:
        _NC_CACHE["nc"] = build(NP, N_CORES)
    return _NC_CACHE["nc"]


def kernel(x_r, x_i, batch_index, state_index, Wa_r, Wa_i, Wd_r, Wd_i,
           **_unused):
    x_r = np.ascontiguousarray(np.asarray(x_r, dtype=np.float32))
    x_i = np.ascontiguousarray(np.asarray(x_i, dtype=np.float32))
    batch_index = np.asarray(batch_index, dtype=np.int32)
    N = x_r.shape[0]
    assert N == N_TOTAL, f"kernel compiled for N={N_TOTAL}, got {N}"

    # group-aligned shard starts
    starts = [0]
    for c in range(1, N_CORES):
        pos = (N * c) // N_CORES
        g = int(batch_index[pos])
        starts.append(int(np.searchsorted(batch_index, g)))
    starts.append(N)
    bases = []
    for c in range(N_CORES):
        b = starts[c]
        if b + NP > N:
            b = N - NP
        assert b >= 0 and starts[c + 1] - b <= NP
        bases.append(b)

    def wpack(Wa, Wd):
        w = np.zeros((DIM, 8), np.float32)
        w[:, 0:6] = np.asarray(Wa, dtype=np.float32).T
        w[:, 6] = np.asarray(Wd, dtype=np.float32)[0]
        w /= DIM
        return w.astype(ml_dtypes.bfloat16)

    w8r = wpack(Wa_r, Wd_r)
    w8i = wpack(Wa_i, Wd_i)
    ident = np.eye(128, dtype=np.float32)

    in_maps = []
    for c in range(N_CORES):
        b = bases[c]
        bip = np.empty(NP + 2, np.int32)
        bip[1:NP + 1] = batch_index[b:b + NP]
        bip[0] = batch_index[b - 1] if b > 0 else -1
        bip[NP + 1] = batch_index[b + NP] if b + NP < N else -2
        in_maps.append(dict(
            xr=x_r[b:b + NP], xi=x_i[b:b + NP],
            w8r=w8r, w8i=w8i, bip=bip, ident=ident,
        ))

    nc = _get_nc()
    res = run_bass_kernel_spmd(nc, in_maps, core_ids=list(range(N_CORES)),
                               trace=False)

    out = np.empty((N, 6), np.float32)
    for c in range(N_CORES):
        lo, hi = starts[c], starts[c + 1]
        b = bases[c]
        out[lo:hi] = res.results[c]["out8"][lo - b:hi - b, 0:6]
    return out



# revision 2
# speedup vs baseline: 4.3092x; 4.3092x over previous
"""Trainium2 Bass kernel for nn_Actor_1580547975181 (segment_reduce), v2.

out[n, a] = log_softmax(action_select)[n, a] + scatter_log_softmax(device_select)[n]
  action_select = (x_r @ Wa_r.T + x_i @ Wa_i.T) / 128     [N, 6]
  device_select = (x_r @ Wd_r[0] + x_i @ Wd_i[0]) / 128   [N]
  groups = sorted batch_index (B=8192 segments over N=1048576 rows)

Sharding: rows split across 8 cores at group boundaries; each core runs a
fixed window of NP=133120 rows.

v2 design notes (cost-model driven; DMA transfer time is a single serial
resource, descriptors >= 512B contiguous on both sides run at full rate and
are charged on OUTPUT bytes - so f32->bf16 cast loads with >= 512B bf16 runs
cost half of the baseline's 256B-run layout):
  * row mapping per 8192-row chunk c: row = c*8192 + p*W + w (W=64; tail
    chunk W=16), i.e. partition p holds W consecutive rows. Loads pull
    8-row-blocks of both tensors into staging [p, (j, u, d)] with 2KB
    contiguous descriptors (full DMA rate on bf16 bytes).
  * transposes move to the PE (128x128 bf16 tiles via identity matmul into
    PSUM, 8 tiles per bank), PSUM->SBUF bf16 copies split across DVE/ACT,
    then per-tile matmuls with the transposed tile stationary and the packed
    [128, 8] weights moving (8-column moving operand; nearly free).
  * v = y[:,:,6] lands directly in scan layout [p, w] (consecutive rows along
    the free axis), so the segmented scan needs no pre-transpose. Per-chunk
    carries are batched: per-chunk packs [128, nblk] are flattened once into
    [1, 128*nblk] global rows, one scan resolves all cross-partition carries,
    one more resolves cross-chunk group sums (groups can span partitions).
  * output written as [p, (w, 8)] f32 -> 2KB contiguous HBM descriptors.
  * gpsimd/Pool engine issues ONLY the cast loads (SWDGE) so the serial DMA
    resource never starves behind compute.
"""

from contextlib import ExitStack

import numpy as np
import ml_dtypes

import concourse.bass as bass
import concourse.tile as tile
from concourse import bacc, mybir
from concourse.bass_utils import run_bass_kernel_spmd

F32 = mybir.dt.float32
BF16 = mybir.dt.bfloat16
I32 = mybir.dt.int32
X8 = True                                  # stage x in fp8e4m3 (else bf16)
XDT = mybir.dt.float8e4 if X8 else mybir.dt.bfloat16
AF = mybir.ActivationFunctionType
OP = mybir.AluOpType
AX = mybir.AxisListType

N_CORES = 8
DIM = 128
N_TOTAL = 1048576
CW = 64                                   # w-slots per full chunk (per partition)
ROWS_PER_CHUNK = 128 * CW                 # 8192
TAIL_W = 16                               # tail chunk w-slots (2048 rows)
NP = 16 * ROWS_PER_CHUNK + 128 * TAIL_W   # 133120
BLK = 8                                   # tiles per transpose/copy block


def _bcast(ap2, n):
    return bass.AP(ap2.tensor, ap2.offset, [*[list(p) for p in ap2.ap], [0, n]])


def _sub(ap, off, dims):
    return bass.AP(ap.tensor, ap.offset + off, dims)


def build(NP, n_devices=N_CORES, phases=5):
    assert NP % ROWS_PER_CHUNK in (0, 128 * TAIL_W)
    nfull = NP // ROWS_PER_CHUNK
    chunks = [(c, CW) for c in range(nfull)]
    if NP % ROWS_PER_CHUNK:
        chunks.append((nfull, TAIL_W))
    nblk = len(chunks)
    nflat = nblk * 128
    ntw = sum(w for _, w in chunks)       # total w-slots (NP // 128)

    nc = bacc.Bacc("TRN2", target_bir_lowering=False, debug=False,
                   num_devices=n_devices)

    xr = nc.dram_tensor("xr", [NP, 128], F32, kind="ExternalInput")
    xi = nc.dram_tensor("xi", [NP, 128], F32, kind="ExternalInput")
    w8r = nc.dram_tensor("w8r", [128, 8], XDT, kind="ExternalInput")
    w8i = nc.dram_tensor("w8i", [128, 8], XDT, kind="ExternalInput")
    bip = nc.dram_tensor("bip", [NP + 2], I32, kind="ExternalInput")
    identf = nc.dram_tensor("identf", [128, 128], F32, kind="ExternalInput")
    identb = nc.dram_tensor("identb", [128, 128], XDT, kind="ExternalInput")
    out8 = nc.dram_tensor("out8", [NP, 8], BF16, kind="ExternalOutput")

    # copy-engine schedule per chunk (16 blocks): DVE-heavy, ACT the rest
    CP_ENG = ["vector", "scalar", "vector", "vector", "scalar", "vector",
              "vector", "scalar", "vector", "scalar", "vector", "vector",
              "scalar", "vector", "vector", "scalar"]

    with tile.TileContext(nc) as tc, ExitStack() as ctx:
        ep = ctx.enter_context
        pconst = ep(tc.tile_pool(name="const", bufs=1))
        ppers = ep(tc.tile_pool(name="pers", bufs=1))
        pstg = ep(tc.tile_pool(name="stg", bufs=2))
        pxt = ep(tc.tile_pool(name="xt", bufs=6))
        pscr = ep(tc.tile_pool(name="scr", bufs=3))
        pbi = ep(tc.tile_pool(name="bi", bufs=2))
        pout = ep(tc.tile_pool(name="outp", bufs=6))
        ppsT = ep(tc.tile_pool(name="psT", bufs=4, space="PSUM"))
        ppsY = ep(tc.tile_pool(name="psY", bufs=2, space="PSUM"))
        ppsS = ep(tc.tile_pool(name="psS", bufs=2, space="PSUM"))

        idf = pconst.tile([128, 128], F32)
        nc.sync.dma_start(idf[:], identf.ap())
        idb = pconst.tile([128, 128], XDT)
        nc.sync.dma_start(idb[:], identb.ap())
        wr_t = pconst.tile([128, 8], XDT)
        nc.sync.dma_start(wr_t[:], w8r.ap())
        wi_t = pconst.tile([128, 8], XDT)
        nc.sync.dma_start(wi_t[:], w8i.ap())

        # warm the PE p-state through the first chunk's load (the p-state
        # model needs a continuous busy stretch; span the ~14us until the
        # first staged chunk is transposable)
        for _ in range(16):
            wps = ppsT.tile([128, BLK * 128 * (2 if X8 else 1)], XDT,
                            tag="pT")
            wa = wps[:]
            for u in range(BLK):
                nc.tensor.transpose(
                    bass.AP(wa.tensor, wa.offset + u * 128 * (2 if X8 else 1),
                            [[wa.ap[0][0], 128], [(2 if X8 else 1), 128]]),
                    idb[:], idb[:])

        stash = ppers.tile([128, ntw * 6], BF16)
        vstage = ppers.tile([128, ntw], F32)
        asumst = ppers.tile([128, ntw], F32)
        Sall = ppers.tile([128, ntw], F32)
        Gall = ppers.tile([128, ntw], F32)
        notEall = ppers.tile([128, ntw], F32)
        maskCall = ppers.tile([128, ntw], F32)
        eAll = ppers.tile([128, ntw], F32)
        m1all = ppers.tile([128, ntw], BF16)
        m2all = ppers.tile([128, ntw], BF16)
        flagEall = ppers.tile([128, ntw], BF16)
        SLpack = ppers.tile([128, nblk], F32)
        nc.vector.memset(notEall[:], 0.0)
        contpack = ppers.tile([128, nblk], F32)
        fullpack = ppers.tile([128, nblk], F32)
        G0pack = ppers.tile([128, nblk], F32)
        cinpack = ppers.tile([128, nblk], F32)
        Fpack = ppers.tile([128, nblk], F32)

        def tw_base(c):
            return c * CW

        def emit_chunk(c, W):
            row0 = c * ROWS_PER_CHUNK
            nb = W // BLK                 # 8-tile blocks per half
            tb = tw_base(c)

            # segment masks from batch_index depend only on bip - emit first
            # so they fill engine idle time and never stall the pipeline.
            # bip[i] = batch_index[base+i-1]
            bi3 = pbi.tile([128, CW + 2], I32, tag="bi")
            nc.sync.dma_start(bi3[:, 0:W + 2],
                              bass.AP(bip, row0, [[W, 128], [1, W + 2]]))
            bcur = bi3[:, 1:W + 1]
            nc.vector.tensor_tensor(maskCall[:, tb:tb + W], bcur,
                                    bi3[:, 0:W], op=OP.is_equal)
            bif = pscr.tile([128, CW + 2], F32, tag="bif")
            nc.vector.tensor_copy(bif[:, 0:W + 2], bi3[:, 0:W + 2])
            nc.vector.tensor_scalar(m1all[:, tb:tb + W], bif[:, 1:W + 1],
                                    bif[:, 1:2], None, op0=OP.is_equal)
            nc.vector.tensor_scalar(m2all[:, tb:tb + W], bif[:, 1:W + 1],
                                    bif[:, W:W + 1], None, op0=OP.is_equal)
            # notE's last column stays 0 (upfront memset): the global reverse
            # scan then cannot leak across chunk boundaries, so one scan
            # covers all chunks
            nc.vector.tensor_tensor(notEall[:, tb:tb + W - 1], bcur[:, 0:W - 1],
                                    bi3[:, 2:W + 1], op=OP.is_equal)
            nc.vector.tensor_tensor(flagEall[:, tb:tb + W], bcur,
                                    bi3[:, 2:W + 2], op=OP.not_equal)
            nc.vector.tensor_tensor(contpack[:, c:c + 1], bi3[:, 1:2],
                                    bi3[:, 0:1], op=OP.is_equal)
            nc.vector.tensor_tensor(fullpack[:, c:c + 1], bi3[:, 1:2],
                                    bi3[:, W:W + 1], op=OP.is_equal)

            # staging: j-blocks of 8 tiles, even j = xr rows, odd j = xi.
            stA = pstg.tile([128, 2 * (CW // BLK), BLK * 128], XDT, tag="stA")
            sa = stA[:]
            pstr = sa.ap[0][0]
            dstR = _sub(sa, 0, [[pstr, 128], [2 * BLK * 128, nb], [1, BLK * 128]])
            dstI = _sub(sa, BLK * 128,
                        [[pstr, 128], [2 * BLK * 128, nb], [1, BLK * 128]])
            srcR = bass.AP(xr, row0 * 128,
                           [[W * 128, 128], [BLK * 128, nb], [1, BLK * 128]])
            srcI = bass.AP(xi, row0 * 128,
                           [[W * 128, 128], [BLK * 128, nb], [1, BLK * 128]])
            nc.gpsimd.dma_start(dstR, srcR)
            nc.gpsimd.dma_start(dstI, srcI)

            y = ppsY.tile([128, CW, 8], F32, tag="y")
            # fp8 PE transposes must write with element step 2 (HW writes
            # 16-bit units); data sits at even bytes, matmuls read stride-2
            ST = 2 if X8 else 1

            def xt_ap(t, u):
                ta = t[:]
                return bass.AP(ta.tensor, ta.offset + u * 128 * ST,
                               [[ta.ap[0][0], 128], [ST, 128]])

            def emit_mms(pair):
                k, br, bi_ = pair
                for u in range(BLK):
                    w = k * BLK + u
                    nc.tensor.matmul(y[:, w, :], xt_ap(br, u),
                                     wr_t[:], start=True, stop=False)
                    nc.tensor.matmul(y[:, w, :], xt_ap(bi_, u),
                                     wi_t[:], start=False, stop=True)

            # matmuls trail the transposes by one block pair so the PE's
            # in-order queue never waits on a fresh PSUM->SBUF copy
            xbuf_r = None
            pend = []
            for j in range(2 * nb):
                pT = ppsT.tile([128, BLK * 128 * ST], XDT, tag="pT")
                for u in range(BLK):
                    nc.tensor.transpose(xt_ap(pT, u),
                                        stA[:, j, u * 128:(u + 1) * 128],
                                        idb[:])
                xb = pxt.tile([128, BLK * 128 * ST], XDT, tag="xT")
                cpo, cpi = xb[:], pT[:]
                if X8:
                    cpo, cpi = cpo.bitcast(BF16), cpi.bitcast(BF16)
                eng = getattr(nc, CP_ENG[j % 16])
                if CP_ENG[j % 16] == "scalar":
                    eng.activation(cpo, cpi, AF.Copy)
                else:
                    eng.tensor_copy(cpo, cpi)
                if j % 2 == 0:
                    xbuf_r = xb
                else:
                    pend.append((j // 2, xbuf_r, xb))
                    if len(pend) >= 2:
                        emit_mms(pend.pop(0))
            for pair in pend:
                emit_mms(pair)

            # y epilogue: only materialize y (actions bf16, v f32);
            # exp/sums/scans run one chunk behind (emit_postchunk)
            st3 = stash[:, tb * 6:(tb + W) * 6].rearrange("p (t a) -> p t a",
                                                          a=6)
            nc.scalar.activation(st3, y[:, 0:W, 0:6], AF.Copy,
                                 scale=1.0 / DIM)
            nc.vector.tensor_scalar(vstage[:, tb:tb + W], y[:, 0:W, 6],
                                    1.0 / DIM, None, op0=OP.mult)

        def emit_postchunk(c, W):
            # exp/action-sums/segment scan for an already-materialized chunk;
            # emitted one chunk late so nothing here stalls the pipeline
            tb = tw_base(c)
            nc.scalar.activation(eAll[:, tb:tb + W], vstage[:, tb:tb + W],
                                 AF.Exp)
            ea = pscr.tile([128, CW * 6], BF16, tag="ea")
            ea3 = ea[:, 0:W * 6].rearrange("p (t a) -> p t a", a=6)
            st3 = stash[:, tb * 6:(tb + W) * 6].rearrange("p (t a) -> p t a",
                                                          a=6)
            nc.scalar.activation(ea3, st3, AF.Exp)
            nc.vector.tensor_reduce(asumst[:, tb:tb + W], ea3, axis=AX.X,
                                    op=OP.add)
            nc.vector.tensor_tensor_scan(Sall[:, tb:tb + W],
                                         maskCall[:, tb:tb + W],
                                         eAll[:, tb:tb + W], 0.0,
                                         op0=OP.mult, op1=OP.add)

        def to_sq(srcpack, tag):
            # [128, nblk] pack -> [nblk, 128] sq layout (flat run order is
            # (chunk, partition) = (sq partition, sq free))
            tp = ppsS.tile([nblk, 128], F32, tag="ps")
            nc.tensor.transpose(tp[:], srcpack[:], idf[:])
            sq = pscr.tile([nblk, 128], F32, tag="sq" + tag)
            nc.vector.tensor_copy(sq[:], tp[:])
            return sq

        def from_sq(sq_ap, dstpack, tag):
            tp = ppsS.tile([128, nblk], F32, tag="ps")
            nc.tensor.transpose(tp[:], sq_ap, idf[0:nblk, 0:nblk])
            nc.vector.tensor_copy(dstpack[:], tp[:])

        def col_to_row(col_ap, tag):
            # [nblk, 1] column -> [1, nblk] row at base partition 0
            tp = ppsS.tile([1, nblk], F32, tag="ps")
            nc.tensor.transpose(tp[:], col_ap, idf[0:nblk, 0:nblk])
            row = pscr.tile([1, nblk], F32, tag="r" + tag)
            nc.vector.tensor_copy(row[:], tp[:])
            return row

        def row_to_col(row_ap, tag):
            tp = ppsS.tile([nblk, 1], F32, tag="ps")
            nc.tensor.transpose(tp[:], row_ap, idf[0:1, 0:1])
            col = pscr.tile([nblk, 1], F32, tag="c" + tag)
            nc.vector.tensor_copy(col[:], tp[:])
            return col

        def emit_global_a():
            # one scan resolves every cross-partition carry (flat order
            # f = c*128 + p matches global row order)
            sa_ = Sall[:]
            nc.vector.tensor_copy(
                SLpack[:, 0:nfull],
                bass.AP(sa_.tensor, sa_.offset + CW - 1,
                        [list(sa_.ap[0]), [CW, nfull]]))
            if ntw > FULL:
                nc.vector.tensor_copy(SLpack[:, nfull:nblk],
                                      Sall[:, ntw - 1:ntw])
            apack = pscr.tile([128, nblk], F32, tag="apk")
            nc.vector.tensor_tensor(apack[:], contpack[:], fullpack[:],
                                    op=OP.mult)
            zsq = pscr.tile([nblk, 128], F32, tag="zsq")
            nc.vector.memset(zsq[:], 0.0)
            slsq = to_sq(SLpack, "sl")
            asq = to_sq(apack, "a")
            csq = to_sq(contpack, "c")
            # in-chunk scan + prefix products of a
            stsq = pscr.tile([nblk, 128], F32, tag="st")
            nc.vector.tensor_tensor_scan(stsq[:], asq[:], slsq[:], 0.0,
                                         op0=OP.mult, op1=OP.add)
            prodA = pscr.tile([nblk, 128], F32, tag="pA")
            nc.vector.tensor_tensor_scan(prodA[:], asq[:], zsq[:], 1.0,
                                         op0=OP.mult, op1=OP.add)
            # cross-chunk carry: cq[c] = prod_c*cq[c-1] + st_last[c]
            lastst = col_to_row(stsq[:, 127:128], "ls")
            lastpr = col_to_row(prodA[:, 127:128], "lp")
            cq = pscr.tile([1, nblk], F32, tag="cq")
            nc.vector.tensor_tensor_scan(cq[:], lastpr[:], lastst[:], 0.0,
                                         op0=OP.mult, op1=OP.add)
            cinrow = pscr.tile([1, nblk], F32, tag="cir")
            nc.vector.tensor_copy(cinrow[:, 1:nblk], cq[:, 0:nblk - 1])
            nc.vector.memset(cinrow[:, 0:1], 0.0)
            cincol = row_to_col(cinrow[:], "ci")
            # global st and cin per run
            stfin = pscr.tile([nblk, 128], F32, tag="sf")
            nc.vector.scalar_tensor_tensor(stfin[:], prodA[:], cincol[:],
                                           stsq[:], op0=OP.mult, op1=OP.add)
            stprev = pscr.tile([nblk, 128], F32, tag="sp")
            nc.vector.tensor_copy(stprev[:, 1:128], stfin[:, 0:127])
            nc.vector.tensor_copy(stprev[:, 0:1], cincol[:])
            cinsq = pscr.tile([nblk, 128], F32, tag="cs")
            nc.vector.tensor_tensor(cinsq[:], csq[:], stprev[:], op=OP.mult)
            from_sq(cinsq[:], cinpack, "ci")
            return asq, csq, zsq

        FULL = nfull * CW

        def _cw_bcast(pack, lo, n, w):
            # [128, n, w] view of pack[:, lo:lo+n] with w-fold broadcast
            pa = pack[:]
            return bass.AP(pa.tensor, pa.offset + lo,
                           [list(pa.ap[0]), [1, n], [0, w]])

        def _pack_bcast_stt(dstall, maskall, pack, baseall):
            # dstall = maskall * bcast(pack) + baseall, via a stride-0 view
            # (tmp rides in eAll, dead after the scan phase)
            m3 = maskall[:, 0:FULL].rearrange("p (c w) -> p c w", w=CW)
            e3 = eAll[:, 0:FULL].rearrange("p (c w) -> p c w", w=CW)
            nc.vector.tensor_tensor(e3, m3, _cw_bcast(pack, 0, nfull, CW),
                                    op=OP.mult)
            nc.vector.tensor_tensor(dstall[:, 0:FULL], eAll[:, 0:FULL],
                                    baseall[:, 0:FULL], op=OP.add)
            if ntw > FULL:
                tw = ntw - FULL
                m3t = maskall[:, FULL:ntw].rearrange("p (c w) -> p c w", w=tw)
                e3t = eAll[:, FULL:ntw].rearrange("p (c w) -> p c w", w=tw)
                nc.vector.tensor_tensor(e3t, m3t,
                                        _cw_bcast(pack, nfull, 1, tw),
                                        op=OP.mult)
                nc.vector.tensor_tensor(dstall[:, FULL:ntw],
                                        eAll[:, FULL:ntw],
                                        baseall[:, FULL:ntw], op=OP.add)

        def emit_fill_all():
            # Sf (into maskCall, dead) = m1*cin + S; t1 (into eAll) = flagE*Sf;
            # one global reverse scan fills G (notE chunk-boundary cols are 0)
            _pack_bcast_stt(maskCall, m1all, cinpack, Sall)
            nc.vector.tensor_tensor(eAll[:], flagEall[:], maskCall[:],
                                    op=OP.mult)
            nc.vector.tensor_tensor_scan(Gall[:, ::-1], notEall[:, ::-1],
                                         eAll[:, ::-1], 0.0,
                                         op0=OP.mult, op1=OP.add)
            ga = Gall[:]
            nc.vector.tensor_copy(
                G0pack[:, 0:nfull],
                bass.AP(ga.tensor, ga.offset, [list(ga.ap[0]), [CW, nfull]]))
            if ntw > FULL:
                nc.vector.tensor_copy(G0pack[:, nfull:nblk],
                                      Gall[:, FULL:FULL + 1])

        def emit_global_b(asq, csq, zsq):
            # backward chain for groups crossing partition-run starts:
            # F[f] = d'[f+1] + a[f+1]*F[f+1] with d' = cont*G0, in sq space:
            # per-chunk reverse scans + a [1, nblk] backward carry level
            g0sq = to_sq(G0pack, "g0")
            dd = pscr.tile([nblk, 128], F32, tag="dd")
            nc.vector.tensor_tensor(dd[:], csq[:], g0sq[:], op=OP.mult)
            # row-local backward partial (carry 0) and suffix products over
            # the shifted sequences alpha = a[.,p+1], beta = d'[.,p+1]
            Fpart = pscr.tile([nblk, 128], F32, tag="Fp")
            nc.vector.tensor_tensor_scan(Fpart[:, 0:127][:, ::-1],
                                         asq[:, 1:128][:, ::-1],
                                         dd[:, 1:128][:, ::-1], 0.0,
                                         op0=OP.mult, op1=OP.add)
            Psuf = pscr.tile([nblk, 128], F32, tag="Ps")
            nc.vector.tensor_tensor_scan(Psuf[:, 0:127][:, ::-1],
                                         asq[:, 1:128][:, ::-1],
                                         zsq[:, 1:128][:, ::-1], 1.0,
                                         op0=OP.mult, op1=OP.add)
            # carry_c = B1[c+1] + B2[c+1]*carry_{c+1},
            # B1 = a0*Fpart0 + d0, B2 = a0*Psuf0
            b1 = pscr.tile([nblk, 1], F32, tag="b1")
            nc.vector.tensor_tensor(b1[:], asq[:, 0:1], Fpart[:, 0:1],
                                    op=OP.mult)
            nc.vector.tensor_tensor(b1[:], b1[:], dd[:, 0:1], op=OP.add)
            b2 = pscr.tile([nblk, 1], F32, tag="b2")
            nc.vector.tensor_tensor(b2[:], asq[:, 0:1], Psuf[:, 0:1],
                                    op=OP.mult)
            b1r = col_to_row(b1[:], "b1")
            b2r = col_to_row(b2[:], "b2")
            crow = pscr.tile([1, nblk], F32, tag="crw")
            nc.vector.memset(crow[:, nblk - 1:nblk], 0.0)
            nc.vector.tensor_tensor_scan(crow[:, 0:nblk - 1][:, ::-1],
                                         b2r[:, 1:nblk][:, ::-1],
                                         b1r[:, 1:nblk][:, ::-1], 0.0,
                                         op0=OP.mult, op1=OP.add)
            ccol = row_to_col(crow[:], "cc")
            # Fsq = Psuf*carry + Fpart; last column is the carry itself
            Fsq = pscr.tile([nblk, 128], F32, tag="Fs")
            nc.vector.scalar_tensor_tensor(Fsq[:, 0:127], Psuf[:, 0:127],
                                           ccol[:], Fpart[:, 0:127],
                                           op0=OP.mult, op1=OP.add)
            nc.vector.tensor_copy(Fsq[:, 127:128], ccol[:])
            from_sq(Fsq[:], Fpack, "F")

        def emit_gf_all():
            # Gf = m2*F + G, written over Sall (dead after the fill phase)
            _pack_bcast_stt(Sall, m2all, Fpack, Gall)

        def emit_rdiff():
            # batched: lg over all rows (into notEall, dead), ln(asum) into
            # Gall (dead), rdiff in place on vstage
            nc.scalar.activation(notEall[:], Sall[:], AF.Ln)
            nc.scalar.activation(Gall[:], asumst[:], AF.Ln)
            nc.vector.tensor_tensor(vstage[:], vstage[:], Gall[:],
                                    op=OP.subtract)
            nc.vector.tensor_tensor(vstage[:], vstage[:], notEall[:],
                                    op=OP.subtract)

        def emit_final(c, W):
            row0 = c * ROWS_PER_CHUNK
            tb = tw_base(c)
            ob = pout.tile([128, CW, 8], BF16, tag="ob")
            st3 = stash[:, tb * 6:(tb + W) * 6].rearrange("p (t a) -> p t a",
                                                          a=6)
            eng = nc.vector if c % 2 == 0 else nc.gpsimd
            eng.tensor_tensor(ob[:, 0:W, 0:6], st3,
                              _bcast(vstage[:, tb:tb + W], 6), op=OP.add)
            dst = bass.AP(out8, row0 * 8, [[W * 8, 128], [1, W * 8]])
            nc.sync.dma_start(dst, ob[:, 0:W, :])

        for ci, (c, W) in enumerate(chunks):
            emit_chunk(c, W)
            if ci >= 1:
                emit_postchunk(*chunks[ci - 1])
        if phases >= 2:
            emit_postchunk(*chunks[-1])
        if phases >= 3:
            asq, csq, zsq = emit_global_a()
            emit_fill_all()
        if phases >= 4:
            emit_global_b(asq, csq, zsq)
            emit_gf_all()
            emit_rdiff()
        if phases >= 5:
            for c, W in chunks:
                emit_final(c, W)

    nc.compile()
    return nc


_NC_CACHE = {}


def _get_nc():
    if "nc" not in _NC_CACHE:
        _NC_CACHE["nc"] = build(NP, N_CORES)
    return _NC_CACHE["nc"]


def kernel(x_r, x_i, batch_index, state_index, Wa_r, Wa_i, Wd_r, Wd_i,
           **_unused):
    x_r = np.ascontiguousarray(np.asarray(x_r, dtype=np.float32))
    x_i = np.ascontiguousarray(np.asarray(x_i, dtype=np.float32))
    batch_index = np.asarray(batch_index, dtype=np.int32)
    N = x_r.shape[0]
    assert N == N_TOTAL, f"kernel compiled for N={N_TOTAL}, got {N}"

    # group-aligned shard starts
    starts = [0]
    for c in range(1, N_CORES):
        pos = (N * c) // N_CORES
        g = int(batch_index[pos])
        starts.append(int(np.searchsorted(batch_index, g)))
    starts.append(N)
    bases = []
    for c in range(N_CORES):
        b = starts[c]
        if b + NP > N:
            b = N - NP
        assert b >= 0 and starts[c + 1] - b <= NP
        bases.append(b)

    def wpack(Wa, Wd):
        # 1/DIM is folded into the on-device y materialization so fp8
        # weights stay in normal range
        w = np.zeros((DIM, 8), np.float32)
        w[:, 0:6] = np.asarray(Wa, dtype=np.float32).T
        w[:, 6] = np.asarray(Wd, dtype=np.float32)[0]
        wdt = ml_dtypes.float8_e4m3 if X8 else ml_dtypes.bfloat16
        return w.astype(wdt)

    w8r = wpack(Wa_r, Wd_r)
    w8i = wpack(Wa_i, Wd_i)
    identf = np.eye(128, dtype=np.float32)
    xdt = ml_dtypes.float8_e4m3 if X8 else ml_dtypes.bfloat16
    identb = np.eye(128, dtype=np.float32).astype(xdt)

    in_maps = []
    for c in range(N_CORES):
        b = bases[c]
        bip = np.empty(NP + 2, np.int32)
        bip[1:NP + 1] = batch_index[b:b + NP]
        bip[0] = batch_index[b - 1] if b > 0 else -1
        bip[NP + 1] = batch_index[b + NP] if b + NP < N else -2
        in_maps.append(dict(
            xr=x_r[b:b + NP], xi=x_i[b:b + NP],
            w8r=w8r, w8i=w8i, bip=bip, identf=identf, identb=identb,
        ))

    nc = _get_nc()
    res = run_bass_kernel_spmd(nc, in_maps, core_ids=list(range(N_CORES)),
                               trace=False)

    out = np.empty((N, 6), np.float32)
    for c in range(N_CORES):
        lo, hi = starts[c], starts[c + 1]
        b = bases[c]
        out[lo:hi] = res.results[c]["out8"][lo - b:hi - b, 0:6].astype(
            np.float32)
    return out
